# revision 26
# baseline (speedup 1.0000x reference)
"""Trainium2 Bass kernel for the batched attention module:

    proj   = input @ W.T + b            # [B, TQ, 2H]
    scores = proj @ enc                 # [B, TQ, S]   (enc: [B, 2H, S], S == 2H)
    attn   = softmax(scores, axis=-1)
    out    = attn @ enc                 # [B, TQ, S]

Sharding: data-parallel over batch, one batch per NeuronCore (8 cores).
All matmuls run as float32r (fp32 stored, fp22 multiplied, fp32 accumulated)
which streams at 1 cycle/row on the PE -- 4x the fp32 rate.

Dataflow per core (batch):
  P1:  projT[d,q] = sum_h WT[h,d] * inputT[h,q]  (+bias), per q-group of 512
  P2:  scores[q,s] (q on partitions) accumulated over 16 d-tiles in PSUM
       softmax stats on the free dim: DVE row-max (negated) -> ACT Exp with
       per-partition bias and accumulated row-sum -> DVE reciprocal.
       E is written in bf16: the PE transposes then run at 1 cyc/row
       (vs 2 for fp32) and the PSUM->SBUF eviction casts are 2x cheaper.
  T:   PE-transpose E=[q,s] in bf16, 4 128x128 blocks packed per PSUM
       bank, evicted with one cast per block-group alternating DVE/ACT
  P3:  out[q,v] = sum_s ET[s,q].T @ enc[s,v], scaled by 1/rowsum on eviction

DMA: the front (inp 2MB + W 8MB + enc 16MB) is striped across both HWDGE
queues (sync + scalar) in need-order so the two streams progress in
parallel; the second q-group's W re-stream rides the otherwise idle sync
queue mid-kernel. The scalar queue carries no DMA after the front, so the
ACT exp/copy instructions never queue behind descriptors.
"""

import sys

import numpy as np

for _p in ("/opt/trn_rl_repo",):
    if _p not in sys.path:
        sys.path.insert(0, _p)

from concourse import bacc, bass, mybir, tile  # noqa: E402
from concourse.bass_utils import run_bass_kernel_spmd  # noqa: E402
from concourse.masks import make_identity  # noqa: E402

F32 = mybir.dt.float32
F32R = mybir.dt.float32r
BF16 = mybir.dt.bfloat16
AF = mybir.ActivationFunctionType
AX = mybir.AxisListType
ALU = mybir.AluOpType

B = 8
TQ = 1024
H = 1024
D = 2 * H  # 2048, proj feature dim == contraction dim of scores
S = 2 * H  # 2048
P = 128

NHT = H // P  # 8  h-tiles
NDT = D // P  # 16 d-tiles
NST = S // P  # 16 s-tiles
NQT = TQ // P  # 8 q-tiles
QG = 512  # q-group width for the proj phase (moving dim >= 256 for f32r rate)
NG = TQ // QG  # 2 groups
QTPG = QG // P  # 4 q-tiles per group
NCH = 512  # moving-dim chunk for scores/out matmuls (one PSUM bank of fp32)
NSC = S // NCH  # 4
TPB = 8  # transposes packed per PSUM bank (8 x [128,128]bf16 = 2KB/partition)
NTG = NST // TPB  # 2 transpose groups per q-tile


def r32(ap):
    return ap.bitcast(F32R)


def build_program(loop_n: int = 1) -> bass.Bass:
    nc = bacc.Bacc(
        "TRN2",
        target_bir_lowering=False,
        debug=False,
        # default 16KB/partition of SWDGE descriptor scratch; we only use
        # HWDGE queues (sync/scalar), so reclaim most of it for tiles
        dynamic_dma_scratch_size=2048,
    )
    # host-side pre-transposed layouts: per SBUF partition the DMA reads one
    # long contiguous row (16KB for inp, 4KB for wt) -- short rows (<=2KB)
    # measured ~105GB/s vs ~250+GB/s for long rows
    inpT = nc.declare_dram_parameter("inpT", [NG, P, NHT, QG], F32, isOutput=False)
    wt = nc.declare_dram_parameter("wt", [P, NDT, NHT, P], F32, isOutput=False)
    enc = nc.declare_dram_parameter("enc", [P, NDT, S], F32, isOutput=False)
    bvec = nc.declare_dram_parameter("bvec", [P, NDT], F32, isOutput=False)
    out = nc.declare_dram_parameter("out", [TQ, S], F32, isOutput=True)

    with tile.TileContext(nc) as tc:
        with (
            tc.tile_pool(name="const", bufs=1) as constp,
            tc.tile_pool(name="inp", bufs=1) as inpp,
            tc.tile_pool(name="wtp", bufs=2) as wtp,
            tc.tile_pool(name="projp", bufs=1) as projp,
            tc.tile_pool(name="ep", bufs=1) as ep,
            tc.tile_pool(name="etp", bufs=2) as etp,
            tc.tile_pool(name="outp", bufs=1) as outp,
            tc.tile_pool(name="statp", bufs=2) as statp,
            tc.tile_pool(name="ps_sc", bufs=1, space="PSUM") as ps_sc,
            tc.tile_pool(name="ps_small", bufs=2, space="PSUM") as ps_small,
            tc.tile_pool(name="ps_out", bufs=2, space="PSUM") as ps_out,
        ):
            ident = constp.tile([P, P], BF16)
            make_identity(nc, ident[:])
            bias_sb = constp.tile([P, NDT], F32)
            nc.sync.dma_start(out=bias_sb[:], in_=bvec[:])

            import contextlib

            loop_ctx = (
                tc.For_i(0, loop_n, 1, hint_engines=(mybir.EngineType.PE,))
                if loop_n > 1
                else contextlib.nullcontext()
            )
            loop_ctx.__enter__()

            def emit_p1_pair(g, dtp, projT, inp_g, dma, per_dt=None):
                """Two d-tiles of the proj phase: one 1MB wt DMA + 2x(8
                matmuls + evict). Pairing halves the number of gated wt
                triggers so the stream stays ahead of the PE."""
                wt_sl = wtp.tile([P, 2, NHT, P], F32R, tag="wt")
                dma.dma_start(
                    out=wt_sl[:], in_=r32(wt[:, 2 * dtp : 2 * dtp + 2])
                )
                for k in range(2):
                    dt_ = 2 * dtp + k
                    pp = ps_small.tile([P, QG], F32, tag="small")
                    for ht in range(NHT):
                        nc.tensor.matmul(
                            pp[:],
                            wt_sl[:, k, ht, :],
                            inp_g[:, ht, :],
                            start=(ht == 0),
                            stop=(ht == NHT - 1),
                        )
                    # DVE, not ACT: activation instrs mixed with the DMA queue
                    # on the scalar engine measured ~40us each on HW
                    nc.vector.tensor_scalar_add(
                        projT[:, dt_, :], pp[:], bias_sb[:, dt_ : dt_ + 1]
                    )
                    if per_dt is not None:
                        per_dt(dt_)

            def emit_inp_load(g, dma):
                inp_g = inpp.tile([P, NHT, QG], F32R, tag="inp")
                dma.dma_start(out=inp_g[:], in_=r32(inpT[g]))
                return inp_g

            def emit_scores_dt(sc, projT, qloc, dt_):
                qs = slice(qloc * P, (qloc + 1) * P)
                for c in range(NSC):
                    cs = slice(c * NCH, (c + 1) * NCH)
                    nc.tensor.matmul(
                        sc[:, cs],
                        projT[:, dt_, qs],
                        enc_sb[dt_][:, cs],
                        start=(dt_ == 0),
                        stop=(dt_ == NDT - 1),
                    )

            def emit_scores(projT, qloc):
                # dt-outer so each enc tile unlocks 4 matmuls as it arrives
                # (4 interleaved PSUM accumulation groups, one per bank).
                sc = ps_sc.tile([P, S], F32, tag="sc")
                for dt_ in range(NDT):
                    emit_scores_dt(sc, projT, qloc, dt_)
                return sc

            def emit_softmax(sc):
                st = statp.tile([P, 4], F32, tag="stat")
                nc.vector.tensor_reduce(
                    st[:, 0:1], sc[:], axis=AX.X, op=ALU.max, negate=True
                )
                E = ep.tile([P, S], BF16, tag="E")
                nc.scalar.activation(
                    E[:],
                    sc[:],
                    AF.Exp,
                    bias=st[:, 0:1],
                    scale=1.0,
                    accum_out=st[:, 1:2],
                )
                nc.vector.reciprocal(st[:, 2:3], st[:, 1:2])
                return E, st

            def emit_transp(E):
                # bf16 transposes, TPB 128x128 blocks packed into one PSUM
                # bank, evicted with a single cast alternating DVE/ACT so
                # neither engine's latency gates the PE.
                ET = etp.tile([P, NST, P], F32R, tag="ET")
                for grp in range(NTG):
                    tp = ps_small.tile([P, TPB, P], BF16, tag="small")
                    for j in range(TPB):
                        s_ = grp * TPB + j
                        nc.tensor.transpose(
                            tp[:, j, :], E[:, s_ * P : (s_ + 1) * P], ident[:]
                        )
                    dst = ET[:, grp * TPB : (grp + 1) * TPB, :]
                    if grp % 2 == 0:
                        nc.vector.tensor_copy(dst, tp[:])
                    else:
                        nc.scalar.copy(dst, tp[:])
                return ET

            def emit_out(ET, st, qt):
                for c in range(NSC):
                    cs = slice(c * NCH, (c + 1) * NCH)
                    po = ps_out.tile([P, NCH], F32, tag="po")
                    for s_ in range(NST):
                        nc.tensor.matmul(
                            po[:],
                            ET[:, s_, :],
                            enc_sb[s_][:, cs],
                            start=(s_ == 0),
                            stop=(s_ == NST - 1),
                        )
                    ob = outp.tile([P, NCH], F32, tag="ob")
                    nc.vector.tensor_scalar_mul(ob[:], po[:], st[:, 2:3])
                    nc.sync.dma_start(
                        out=out[qt * P : (qt + 1) * P, cs], in_=ob[:]
                    )

            # Software-pipelined emission: PE order per steady-state q-tile is
            # transp(i), [P1(g+1) at group boundary], scores(i+1), out(i) -- the
            # softmax of i+1 runs on DVE/ACT while PE is busy with out(i).
            # Front: interleave P1(0), enc loads, and scores(0) at d-tile
            # granularity. The front is DMA-bound (inp 2MB + W 8MB + enc 16MB
            # must land); striping wt[dt]/enc[dt] across BOTH HWDGE queues in
            # need-order lets the two streams land in parallel while PE chews
            # P1 and scores(0).
            # Queue split: enc rides sync (ungated, free-running), wt+inp
            # ride scalar (slot-gated, stop-and-go). Mixing them on one
            # queue head-of-line blocks the enc stream behind a gated wt
            # trigger.
            _mark(nc, "front")
            projs = {}
            inp_g0 = emit_inp_load(0, nc.scalar)
            projT0 = projp.tile([P, NDT, QG], F32R, tag="projT")
            projs[0] = projT0
            cur_sc = ps_sc.tile([P, S], F32, tag="sc")
            # enc resident tile, loaded in 4 big DMAs (32KB contiguous per
            # partition each) on the free-running sync queue
            enc_big = constp.tile([P, NDT, S], F32R, tag="enc")
            enc_sb = [enc_big[:, dt_, :] for dt_ in range(NDT)]
            # artificial write-after-write gate: the first enc DMA starts
            # only after inp has fully landed, so the 2MB inp load isn't
            # starved to ~100GB/s by the 4MB enc transfers (P1 is the only
            # PE work available for the first ~30us of the front)
            nc.vector.tensor_copy(
                enc_big[0:1, 0:NDT:4, 0:1], inp_g0[0:1, 0:NHT:2, 0:1]
            )

            def front_dt(dt_):
                emit_scores_dt(cur_sc, projs[0], 0, dt_)

            for dtp in range(NDT // 2):
                if dtp % 2 == 0:
                    gi = dtp // 2
                    nc.sync.dma_start(
                        out=enc_big[:, 4 * gi : 4 * gi + 4, :],
                        in_=r32(enc[:, 4 * gi : 4 * gi + 4, :]),
                    )
                emit_p1_pair(0, dtp, projs[0], inp_g0, nc.scalar, per_dt=front_dt)
            # softmax(qt+1) is emitted right after scores(qt+1), BEFORE
            # out(qt): the DVE reduce-max runs at the start of out(qt)'s PE
            # window instead of queueing behind its eviction scales, so
            # transp(qt+1) never stalls on E. The last q-tile's softmax is
            # likewise hidden behind out(6).
            _mark(nc, "softmax(0)")
            E_cur, st_cur = emit_softmax(cur_sc)
            for qt in range(NQT):
                _mark(nc, f"transp({qt})")
                ET = emit_transp(E_cur)
                nxt = qt + 1
                st_next = None
                if nxt < NQT:
                    g, qloc = divmod(nxt, QTPG)
                    if qloc == 0:
                        # group boundary: stream W for group g, interleaving
                        # the next q-tile's scores at d-tile granularity so
                        # the PE never head-of-line blocks on the W DMA
                        _mark(nc, f"P1({g})+scores({nxt})")
                        inp_g = emit_inp_load(g, nc.sync)
                        projT = projp.tile([P, NDT, QG], F32R, tag="projT")
                        projs[g] = projT
                        sc_new = ps_sc.tile([P, S], F32, tag="sc")
                        cur_sc = sc_new
                        for dtp in range(NDT // 2):
                            emit_p1_pair(
                                g,
                                dtp,
                                projT,
                                inp_g,
                                nc.sync,
                                per_dt=lambda dt_: emit_scores_dt(
                                    sc_new, projT, 0, dt_
                                ),
                            )
                    else:
                        _mark(nc, f"scores({nxt})")
                        cur_sc = emit_scores(projs[g], qloc)
                    _mark(nc, f"softmax({nxt})")
                    E_cur, st_next = emit_softmax(cur_sc)
                _mark(nc, f"out({qt})")
                emit_out(ET, st_cur, qt)
                st_cur = st_next
            _mark(nc, "end")
            loop_ctx.__exit__(None, None, None)

    nc.compile()
    return nc


PHASES = []  # (instruction id, label) marks populated during build, for tsim


def _mark(nc, label):
    nm = nc.get_next_instruction_name()  # burns one name; fine
    PHASES.append((int(nm.split("-")[1]), label))


_NC_CACHE = {}


def _get_program(loop_n: int = 1) -> bass.Bass:
    if loop_n not in _NC_CACHE:
        PHASES.clear()
        _NC_CACHE[loop_n] = build_program(loop_n)
    return _NC_CACHE[loop_n]


def _prep_in_maps(input, encoder_output, W, b):
    input = np.ascontiguousarray(input, dtype=np.float32)
    encoder_output = np.ascontiguousarray(encoder_output, dtype=np.float32)
    W = np.ascontiguousarray(W, dtype=np.float32)
    b = np.ascontiguousarray(b, dtype=np.float32)

    # inpT[b, g, p, ht, q] = input[b, g*QG+q, ht*P+p]  -- per partition p the
    # DMA row [NHT, QG] is contiguous (16KB)
    inpT = np.ascontiguousarray(
        input.reshape(B, NG, QG, NHT, P).transpose(0, 1, 4, 3, 2)
    )
    # wt[p, dt, ht, dj] = W[dt*P+dj, ht*P+p] -- per partition p the whole
    # [NDT, NHT, P] span is contiguous (64KB), so paired-dt DMAs read 8KB
    # contiguous per partition
    wt = np.ascontiguousarray(
        W.reshape(NDT, P, NHT, P).transpose(3, 0, 2, 1)
    )
    # enc_pre[b, p, dt, s] = enc[b, dt*P+p, s] -- 128KB contiguous/partition
    encP = np.ascontiguousarray(
        encoder_output.reshape(B, NDT, P, S).transpose(0, 2, 1, 3)
    )
    bvec = np.ascontiguousarray(b.reshape(NDT, P).T)  # [P, NDT]

    return [
        {"inpT": inpT[i], "wt": wt, "enc": encP[i], "bvec": bvec}
        for i in range(B)
    ]


def run(input, encoder_output, W, b, trace=False, loop_n=1):
    """Returns (out [B, TQ, S] float32, BassKernelResults)."""
    nc = _get_program(loop_n)
    in_maps = _prep_in_maps(input, encoder_output, W, b)
    res = run_bass_kernel_spmd(nc, in_maps, list(range(B)), trace=trace)
    out = np.stack([np.asarray(res.results[i]["out"]) for i in range(B)])
    return out, res


def kernel(input, encoder_output, W, b):
    out, _ = run(input, encoder_output, W, b, trace=False)
    return out


# revision 28
# speedup vs baseline: 1.0056x; 1.0056x over previous
"""Trainium2 Bass kernel for the batched attention module:

    proj   = input @ W.T + b            # [B, TQ, 2H]
    scores = proj @ enc                 # [B, TQ, S]   (enc: [B, 2H, S], S == 2H)
    attn   = softmax(scores, axis=-1)
    out    = attn @ enc                 # [B, TQ, S]

Sharding: data-parallel over batch, one batch per NeuronCore (8 cores).
All matmuls run as float32r (fp32 stored, fp22 multiplied, fp32 accumulated)
which streams at 1 cycle/row on the PE -- 4x the fp32 rate.

Dataflow per core (batch):
  P1:  projT[d,q] = sum_h WT[h,d] * inputT[h,q]  (+bias), per q-group of 512
  P2:  scores[q,s] (q on partitions) accumulated over 16 d-tiles in PSUM
       softmax stats on the free dim: DVE row-max (negated) -> ACT Exp with
       per-partition bias and accumulated row-sum -> DVE reciprocal.
       E is written in bf16: the PE transposes then run at 1 cyc/row
       (vs 2 for fp32) and the PSUM->SBUF eviction casts are 2x cheaper.
  T:   PE-transpose E=[q,s] in bf16, 4 128x128 blocks packed per PSUM
       bank, evicted with one cast per block-group alternating DVE/ACT
  P3:  out[q,v] = sum_s ET[s,q].T @ enc[s,v], scaled by 1/rowsum on eviction

DMA: the front (inp 2MB + W 8MB + enc 16MB) is striped across both HWDGE
queues (sync + scalar) in need-order so the two streams progress in
parallel; the second q-group's W re-stream rides the otherwise idle sync
queue mid-kernel. The scalar queue carries no DMA after the front, so the
ACT exp/copy instructions never queue behind descriptors.
"""

import sys

import numpy as np

for _p in ("/opt/trn_rl_repo",):
    if _p not in sys.path:
        sys.path.insert(0, _p)

from concourse import bacc, bass, mybir, tile  # noqa: E402
from concourse.bass_utils import run_bass_kernel_spmd  # noqa: E402
from concourse.masks import make_identity  # noqa: E402

F32 = mybir.dt.float32
F32R = mybir.dt.float32r
BF16 = mybir.dt.bfloat16
AF = mybir.ActivationFunctionType
AX = mybir.AxisListType
ALU = mybir.AluOpType

B = 8
TQ = 1024
H = 1024
D = 2 * H  # 2048, proj feature dim == contraction dim of scores
S = 2 * H  # 2048
P = 128

NHT = H // P  # 8  h-tiles
NDT = D // P  # 16 d-tiles
NST = S // P  # 16 s-tiles
NQT = TQ // P  # 8 q-tiles
QG = 512  # q-group width for the proj phase (moving dim >= 256 for f32r rate)
NG = TQ // QG  # 2 groups
QTPG = QG // P  # 4 q-tiles per group
NCH = 512  # moving-dim chunk for scores/out matmuls (one PSUM bank of fp32)
NSC = S // NCH  # 4
TPB = 8  # transposes packed per PSUM bank (8 x [128,128]bf16 = 2KB/partition)
NTG = NST // TPB  # 2 transpose groups per q-tile


def r32(ap):
    return ap.bitcast(F32R)


def build_program(loop_n: int = 1) -> bass.Bass:
    nc = bacc.Bacc(
        "TRN2",
        target_bir_lowering=False,
        debug=False,
        # default 16KB/partition of SWDGE descriptor scratch; we only use
        # HWDGE queues (sync/scalar), so reclaim most of it for tiles
        dynamic_dma_scratch_size=2048,
    )
    # host-side pre-transposed layouts: per SBUF partition the DMA reads one
    # long contiguous row (16KB for inp, 4KB for wt) -- short rows (<=2KB)
    # measured ~105GB/s vs ~250+GB/s for long rows
    inpT = nc.declare_dram_parameter("inpT", [NG, P, NHT, QG], F32, isOutput=False)
    wt = nc.declare_dram_parameter("wt", [P, NDT, NHT, P], F32, isOutput=False)
    enc = nc.declare_dram_parameter("enc", [P, NDT, S], F32, isOutput=False)
    bvec = nc.declare_dram_parameter("bvec", [P, NDT], F32, isOutput=False)
    out = nc.declare_dram_parameter("out", [TQ, S], F32, isOutput=True)

    with tile.TileContext(nc) as tc:
        with (
            tc.tile_pool(name="const", bufs=1) as constp,
            tc.tile_pool(name="inp", bufs=1) as inpp,
            tc.tile_pool(name="wtp", bufs=2) as wtp,
            tc.tile_pool(name="projp", bufs=1) as projp,
            tc.tile_pool(name="ep", bufs=1) as ep,
            tc.tile_pool(name="etp", bufs=2) as etp,
            tc.tile_pool(name="outp", bufs=1) as outp,
            tc.tile_pool(name="statp", bufs=2) as statp,
            tc.tile_pool(name="ps_sc", bufs=1, space="PSUM") as ps_sc,
            tc.tile_pool(name="ps_small", bufs=2, space="PSUM") as ps_small,
            tc.tile_pool(name="ps_out", bufs=2, space="PSUM") as ps_out,
        ):
            ident = constp.tile([P, P], BF16)
            make_identity(nc, ident[:])
            bias_sb = constp.tile([P, NDT], F32)
            nc.sync.dma_start(out=bias_sb[:], in_=bvec[:])

            import contextlib

            loop_ctx = (
                tc.For_i(0, loop_n, 1, hint_engines=(mybir.EngineType.PE,))
                if loop_n > 1
                else contextlib.nullcontext()
            )
            loop_ctx.__enter__()

            def emit_p1_pair(g, dtp, projT, inp_g, dma, per_dt=None):
                """Two d-tiles of the proj phase: one 1MB wt DMA + 2x(8
                matmuls + evict). Pairing halves the number of gated wt
                triggers so the stream stays ahead of the PE."""
                wt_sl = wtp.tile([P, 2, NHT, P], F32R, tag="wt")
                dma.dma_start(
                    out=wt_sl[:], in_=r32(wt[:, 2 * dtp : 2 * dtp + 2])
                )
                for k in range(2):
                    dt_ = 2 * dtp + k
                    pp = ps_small.tile([P, QG], F32, tag="small")
                    for ht in range(NHT):
                        nc.tensor.matmul(
                            pp[:],
                            wt_sl[:, k, ht, :],
                            inp_g[:, ht, :],
                            start=(ht == 0),
                            stop=(ht == NHT - 1),
                        )
                    # DVE, not ACT: activation instrs mixed with the DMA queue
                    # on the scalar engine measured ~40us each on HW
                    nc.vector.tensor_scalar_add(
                        projT[:, dt_, :], pp[:], bias_sb[:, dt_ : dt_ + 1]
                    )
                    if per_dt is not None:
                        per_dt(dt_)

            def emit_inp_load(g, dma):
                inp_g = inpp.tile([P, NHT, QG], F32R, tag="inp")
                dma.dma_start(out=inp_g[:], in_=r32(inpT[g]))
                return inp_g

            def emit_scores_dt(sc, projT, qloc, dt_):
                qs = slice(qloc * P, (qloc + 1) * P)
                for c in range(NSC):
                    cs = slice(c * NCH, (c + 1) * NCH)
                    nc.tensor.matmul(
                        sc[:, cs],
                        projT[:, dt_, qs],
                        enc_sb[dt_][:, cs],
                        start=(dt_ == 0),
                        stop=(dt_ == NDT - 1),
                    )

            def emit_scores(projT, qloc):
                # dt-outer so each enc tile unlocks 4 matmuls as it arrives
                # (4 interleaved PSUM accumulation groups, one per bank).
                sc = ps_sc.tile([P, S], F32, tag="sc")
                for dt_ in range(NDT):
                    emit_scores_dt(sc, projT, qloc, dt_)
                return sc

            def emit_softmax(sc):
                st = statp.tile([P, 4], F32, tag="stat")
                nc.vector.tensor_reduce(
                    st[:, 0:1], sc[:], axis=AX.X, op=ALU.max, negate=True
                )
                E = ep.tile([P, S], BF16, tag="E")
                nc.scalar.activation(
                    E[:],
                    sc[:],
                    AF.Exp,
                    bias=st[:, 0:1],
                    scale=1.0,
                    accum_out=st[:, 1:2],
                )
                nc.vector.reciprocal(st[:, 2:3], st[:, 1:2])
                return E, st

            def emit_transp(E):
                # bf16 transposes, TPB 128x128 blocks packed into one PSUM
                # bank, evicted with a single cast alternating DVE/ACT so
                # neither engine's latency gates the PE.
                ET = etp.tile([P, NST, P], F32R, tag="ET")
                for grp in range(NTG):
                    tp = ps_small.tile([P, TPB, P], BF16, tag="small")
                    for j in range(TPB):
                        s_ = grp * TPB + j
                        nc.tensor.transpose(
                            tp[:, j, :], E[:, s_ * P : (s_ + 1) * P], ident[:]
                        )
                    dst = ET[:, grp * TPB : (grp + 1) * TPB, :]
                    if grp % 2 == 0:
                        nc.vector.tensor_copy(dst, tp[:])
                    else:
                        nc.scalar.copy(dst, tp[:])
                return ET

            def emit_out(ET, st, qt):
                for c in range(NSC):
                    cs = slice(c * NCH, (c + 1) * NCH)
                    po = ps_out.tile([P, NCH], F32, tag="po")
                    for s_ in range(NST):
                        nc.tensor.matmul(
                            po[:],
                            ET[:, s_, :],
                            enc_sb[s_][:, cs],
                            start=(s_ == 0),
                            stop=(s_ == NST - 1),
                        )
                    ob = outp.tile([P, NCH], F32, tag="ob")
                    nc.vector.tensor_scalar_mul(ob[:], po[:], st[:, 2:3])
                    nc.sync.dma_start(
                        out=out[qt * P : (qt + 1) * P, cs], in_=ob[:]
                    )

            # Software-pipelined emission: PE order per steady-state q-tile is
            # transp(i), [P1(g+1) at group boundary], scores(i+1), out(i) -- the
            # softmax of i+1 runs on DVE/ACT while PE is busy with out(i).
            # Front: interleave P1(0), enc loads, and scores(0) at d-tile
            # granularity. The front is DMA-bound (inp 2MB + W 8MB + enc 16MB
            # must land); striping wt[dt]/enc[dt] across BOTH HWDGE queues in
            # need-order lets the two streams land in parallel while PE chews
            # P1 and scores(0).
            # Queue split: enc rides sync (ungated, free-running), wt+inp
            # ride scalar (slot-gated, stop-and-go). Mixing them on one
            # queue head-of-line blocks the enc stream behind a gated wt
            # trigger.
            _mark(nc, "front")
            projs = {}
            inp_g0 = emit_inp_load(0, nc.scalar)
            projT0 = projp.tile([P, NDT, QG], F32R, tag="projT")
            projs[0] = projT0
            cur_sc = ps_sc.tile([P, S], F32, tag="sc")
            # enc resident tile, loaded in 4 big DMAs (32KB contiguous per
            # partition each) on the free-running sync queue
            enc_big = constp.tile([P, NDT, S], F32R, tag="enc")
            enc_sb = [enc_big[:, dt_, :] for dt_ in range(NDT)]
            # artificial write-after-write gate: the first enc DMA starts
            # only after inp has fully landed, so the 2MB inp load isn't
            # starved to ~100GB/s by the 4MB enc transfers (P1 is the only
            # PE work available for the first ~30us of the front)
            nc.vector.tensor_copy(
                enc_big[0:1, 0:NDT:4, 0:1], inp_g0[0:1, 0:NHT:2, 0:1]
            )

            def front_dt(dt_):
                emit_scores_dt(cur_sc, projs[0], 0, dt_)

            for dtp in range(NDT // 2):
                if dtp % 2 == 0:
                    gi = dtp // 2
                    nc.sync.dma_start(
                        out=enc_big[:, 4 * gi : 4 * gi + 4, :],
                        in_=r32(enc[:, 4 * gi : 4 * gi + 4, :]),
                    )
                emit_p1_pair(0, dtp, projs[0], inp_g0, nc.scalar, per_dt=front_dt)
            # prefetch group 1's inp right after the front: the dummy copy
            # chains it behind the last enc group (so it doesn't steal front
            # bandwidth) and behind P1(0)'s reads of the shared slot
            nc.vector.tensor_copy(inp_g0[0:1, 0, 0:1], enc_big[0:1, NDT - 1, 0:1])
            inp_g1 = emit_inp_load(1, nc.sync)
            # softmax(qt+1) is emitted right after scores(qt+1), BEFORE
            # out(qt): the DVE reduce-max runs at the start of out(qt)'s PE
            # window instead of queueing behind its eviction scales, so
            # transp(qt+1) never stalls on E. The last q-tile's softmax is
            # likewise hidden behind out(6).
            _mark(nc, "softmax(0)")
            E_cur, st_cur = emit_softmax(cur_sc)
            for qt in range(NQT):
                _mark(nc, f"transp({qt})")
                ET = emit_transp(E_cur)
                nxt = qt + 1
                st_next = None
                if nxt < NQT:
                    g, qloc = divmod(nxt, QTPG)
                    if qloc == 0:
                        # group boundary: stream W for group g, interleaving
                        # the next q-tile's scores at d-tile granularity so
                        # the PE never head-of-line blocks on the W DMA
                        _mark(nc, f"P1({g})+scores({nxt})")
                        inp_g = inp_g1
                        projT = projp.tile([P, NDT, QG], F32R, tag="projT")
                        projs[g] = projT
                        sc_new = ps_sc.tile([P, S], F32, tag="sc")
                        cur_sc = sc_new
                        for dtp in range(NDT // 2):
                            emit_p1_pair(
                                g,
                                dtp,
                                projT,
                                inp_g,
                                nc.sync,
                                per_dt=lambda dt_: emit_scores_dt(
                                    sc_new, projT, 0, dt_
                                ),
                            )
                    else:
                        _mark(nc, f"scores({nxt})")
                        cur_sc = emit_scores(projs[g], qloc)
                    _mark(nc, f"softmax({nxt})")
                    E_cur, st_next = emit_softmax(cur_sc)
                _mark(nc, f"out({qt})")
                emit_out(ET, st_cur, qt)
                st_cur = st_next
            _mark(nc, "end")
            loop_ctx.__exit__(None, None, None)

    nc.compile()
    return nc


PHASES = []  # (instruction id, label) marks populated during build, for tsim


def _mark(nc, label):
    nm = nc.get_next_instruction_name()  # burns one name; fine
    PHASES.append((int(nm.split("-")[1]), label))


_NC_CACHE = {}


def _get_program(loop_n: int = 1) -> bass.Bass:
    if loop_n not in _NC_CACHE:
        PHASES.clear()
        _NC_CACHE[loop_n] = build_program(loop_n)
    return _NC_CACHE[loop_n]


def _prep_in_maps(input, encoder_output, W, b):
    input = np.ascontiguousarray(input, dtype=np.float32)
    encoder_output = np.ascontiguousarray(encoder_output, dtype=np.float32)
    W = np.ascontiguousarray(W, dtype=np.float32)
    b = np.ascontiguousarray(b, dtype=np.float32)

    # inpT[b, g, p, ht, q] = input[b, g*QG+q, ht*P+p]  -- per partition p the
    # DMA row [NHT, QG] is contiguous (16KB)
    inpT = np.ascontiguousarray(
        input.reshape(B, NG, QG, NHT, P).transpose(0, 1, 4, 3, 2)
    )
    # wt[p, dt, ht, dj] = W[dt*P+dj, ht*P+p] -- per partition p the whole
    # [NDT, NHT, P] span is contiguous (64KB), so paired-dt DMAs read 8KB
    # contiguous per partition
    wt = np.ascontiguousarray(
        W.reshape(NDT, P, NHT, P).transpose(3, 0, 2, 1)
    )
    # enc_pre[b, p, dt, s] = enc[b, dt*P+p, s] -- 128KB contiguous/partition
    encP = np.ascontiguousarray(
        encoder_output.reshape(B, NDT, P, S).transpose(0, 2, 1, 3)
    )
    bvec = np.ascontiguousarray(b.reshape(NDT, P).T)  # [P, NDT]

    return [
        {"inpT": inpT[i], "wt": wt, "enc": encP[i], "bvec": bvec}
        for i in range(B)
    ]


def run(input, encoder_output, W, b, trace=False, loop_n=1):
    """Returns (out [B, TQ, S] float32, BassKernelResults)."""
    nc = _get_program(loop_n)
    in_maps = _prep_in_maps(input, encoder_output, W, b)
    res = run_bass_kernel_spmd(nc, in_maps, list(range(B)), trace=trace)
    out = np.stack([np.asarray(res.results[i]["out"]) for i in range(B)])
    return out, res


def kernel(input, encoder_output, W, b):
    out, _ = run(input, encoder_output, W, b, trace=False)
    return out


# revision 30
# speedup vs baseline: 1.0070x; 1.0014x over previous
"""Trainium2 Bass kernel for the batched attention module:

    proj   = input @ W.T + b            # [B, TQ, 2H]
    scores = proj @ enc                 # [B, TQ, S]   (enc: [B, 2H, S], S == 2H)
    attn   = softmax(scores, axis=-1)
    out    = attn @ enc                 # [B, TQ, S]

Sharding: data-parallel over batch, one batch per NeuronCore (8 cores).
All matmuls run as float32r (fp32 stored, fp22 multiplied, fp32 accumulated)
which streams at 1 cycle/row on the PE -- 4x the fp32 rate.

Dataflow per core (batch):
  P1:  projT[d,q] = sum_h WT[h,d] * inputT[h,q]  (+bias), per q-group of 512
  P2:  scores[q,s] (q on partitions) accumulated over 16 d-tiles in PSUM
       softmax stats on the free dim: DVE row-max (negated) -> ACT Exp with
       per-partition bias and accumulated row-sum -> DVE reciprocal.
       E is written in bf16: the PE transposes then run at 1 cyc/row
       (vs 2 for fp32) and the PSUM->SBUF eviction casts are 2x cheaper.
  T:   PE-transpose E=[q,s] in bf16, 4 128x128 blocks packed per PSUM
       bank, evicted with one cast per block-group alternating DVE/ACT
  P3:  out[q,v] = sum_s ET[s,q].T @ enc[s,v], scaled by 1/rowsum on eviction

DMA: the front (inp 2MB + W 8MB + enc 16MB) is striped across both HWDGE
queues (sync + scalar) in need-order so the two streams progress in
parallel; the second q-group's W re-stream rides the otherwise idle sync
queue mid-kernel. The scalar queue carries no DMA after the front, so the
ACT exp/copy instructions never queue behind descriptors.
"""

import sys

import numpy as np

for _p in ("/opt/trn_rl_repo",):
    if _p not in sys.path:
        sys.path.insert(0, _p)

from concourse import bacc, bass, mybir, tile  # noqa: E402
from concourse.bass_utils import run_bass_kernel_spmd  # noqa: E402
from concourse.masks import make_identity  # noqa: E402

F32 = mybir.dt.float32
F32R = mybir.dt.float32r
BF16 = mybir.dt.bfloat16
AF = mybir.ActivationFunctionType
AX = mybir.AxisListType
ALU = mybir.AluOpType

B = 8
TQ = 1024
H = 1024
D = 2 * H  # 2048, proj feature dim == contraction dim of scores
S = 2 * H  # 2048
P = 128

NHT = H // P  # 8  h-tiles
NDT = D // P  # 16 d-tiles
NST = S // P  # 16 s-tiles
NQT = TQ // P  # 8 q-tiles
QG = 512  # q-group width for the proj phase (moving dim >= 256 for f32r rate)
NG = TQ // QG  # 2 groups
QTPG = QG // P  # 4 q-tiles per group
NCH = 512  # moving-dim chunk for scores/out matmuls (one PSUM bank of fp32)
NSC = S // NCH  # 4
TPB = 8  # transposes packed per PSUM bank (8 x [128,128]bf16 = 2KB/partition)
NTG = NST // TPB  # 2 transpose groups per q-tile


def r32(ap):
    return ap.bitcast(F32R)


def build_program(loop_n: int = 1) -> bass.Bass:
    nc = bacc.Bacc(
        "TRN2",
        target_bir_lowering=False,
        debug=False,
        # default 16KB/partition of SWDGE descriptor scratch; we only use
        # HWDGE queues (sync/scalar), so reclaim most of it for tiles
        dynamic_dma_scratch_size=2048,
    )
    # host-side pre-transposed layouts: per SBUF partition the DMA reads one
    # long contiguous row (16KB for inp, 4KB for wt) -- short rows (<=2KB)
    # measured ~105GB/s vs ~250+GB/s for long rows
    inpT = nc.declare_dram_parameter("inpT", [NG, P, NHT, QG], F32, isOutput=False)
    wt = nc.declare_dram_parameter("wt", [P, NDT, NHT, P], F32, isOutput=False)
    enc = nc.declare_dram_parameter("enc", [P, NDT, S], F32, isOutput=False)
    bvec = nc.declare_dram_parameter("bvec", [P, NDT], F32, isOutput=False)
    out = nc.declare_dram_parameter("out", [TQ, S], F32, isOutput=True)

    with tile.TileContext(nc) as tc:
        with (
            tc.tile_pool(name="const", bufs=1) as constp,
            tc.tile_pool(name="inp", bufs=1) as inpp,
            tc.tile_pool(name="wtp", bufs=2) as wtp,
            tc.tile_pool(name="projp", bufs=1) as projp,
            tc.tile_pool(name="ep", bufs=1) as ep,
            tc.tile_pool(name="etp", bufs=2) as etp,
            tc.tile_pool(name="outp", bufs=1) as outp,
            tc.tile_pool(name="statp", bufs=2) as statp,
            tc.tile_pool(name="ps_sc", bufs=1, space="PSUM") as ps_sc,
            tc.tile_pool(name="ps_small", bufs=2, space="PSUM") as ps_small,
            tc.tile_pool(name="ps_out", bufs=2, space="PSUM") as ps_out,
        ):
            ident = constp.tile([P, P], BF16)
            make_identity(nc, ident[:])
            bias_sb = constp.tile([P, NDT], F32)
            nc.sync.dma_start(out=bias_sb[:], in_=bvec[:])

            import contextlib

            loop_ctx = (
                tc.For_i(0, loop_n, 1, hint_engines=(mybir.EngineType.PE,))
                if loop_n > 1
                else contextlib.nullcontext()
            )
            loop_ctx.__enter__()

            def emit_p1_pair(g, dtp, projT, inp_g, dma, per_dt=None):
                """Two d-tiles of the proj phase: one 1MB wt DMA + 2x(8
                matmuls + evict). Pairing halves the number of gated wt
                triggers so the stream stays ahead of the PE."""
                wt_sl = wtp.tile([P, 2, NHT, P], F32R, tag="wt")
                dma.dma_start(
                    out=wt_sl[:], in_=r32(wt[:, 2 * dtp : 2 * dtp + 2])
                )
                for k in range(2):
                    dt_ = 2 * dtp + k
                    pp = ps_small.tile([P, QG], F32, tag="small")
                    for ht in range(NHT):
                        nc.tensor.matmul(
                            pp[:],
                            wt_sl[:, k, ht, :],
                            inp_g[:, ht, :],
                            start=(ht == 0),
                            stop=(ht == NHT - 1),
                        )
                    # DVE, not ACT: activation instrs mixed with the DMA queue
                    # on the scalar engine measured ~40us each on HW
                    nc.vector.tensor_scalar_add(
                        projT[:, dt_, :], pp[:], bias_sb[:, dt_ : dt_ + 1]
                    )
                    if per_dt is not None:
                        per_dt(dt_)

            def emit_inp_load(g, dma):
                inp_g = inpp.tile([P, NHT, QG], F32R, tag="inp")
                dma.dma_start(out=inp_g[:], in_=r32(inpT[g]))
                return inp_g

            def emit_scores_dt(sc, projT, qloc, dt_):
                qs = slice(qloc * P, (qloc + 1) * P)
                for c in range(NSC):
                    cs = slice(c * NCH, (c + 1) * NCH)
                    nc.tensor.matmul(
                        sc[:, cs],
                        projT[:, dt_, qs],
                        enc_sb[dt_][:, cs],
                        start=(dt_ == 0),
                        stop=(dt_ == NDT - 1),
                    )

            def emit_scores(projT, qloc):
                # dt-outer so each enc tile unlocks 4 matmuls as it arrives
                # (4 interleaved PSUM accumulation groups, one per bank).
                sc = ps_sc.tile([P, S], F32, tag="sc")
                for dt_ in range(NDT):
                    emit_scores_dt(sc, projT, qloc, dt_)
                return sc

            def emit_softmax(sc):
                # split into s-halves: the first transposes only gate on the
                # first exp half, shortening the exposed chain after scores
                st = statp.tile([P, 8], F32, tag="stat")
                H2 = S // 2
                nc.vector.tensor_reduce(
                    st[:, 0:1], sc[:, 0:H2], axis=AX.X, op=ALU.max, negate=True
                )
                nc.vector.tensor_reduce(
                    st[:, 1:2], sc[:, H2:], axis=AX.X, op=ALU.max, negate=True
                )
                # -max_total = min(-max_lo, -max_hi)
                nc.vector.scalar_tensor_tensor(
                    st[:, 2:3], st[:, 0:1], 0.0, st[:, 1:2], ALU.add, ALU.min
                )
                E = ep.tile([P, S], BF16, tag="E")
                nc.scalar.activation(
                    E[:, 0:H2],
                    sc[:, 0:H2],
                    AF.Exp,
                    bias=st[:, 2:3],
                    scale=1.0,
                    accum_out=st[:, 3:4],
                )
                nc.scalar.activation(
                    E[:, H2:],
                    sc[:, H2:],
                    AF.Exp,
                    bias=st[:, 2:3],
                    scale=1.0,
                    accum_out=st[:, 4:5],
                )
                nc.vector.scalar_tensor_tensor(
                    st[:, 5:6], st[:, 3:4], 0.0, st[:, 4:5], ALU.add, ALU.add
                )
                nc.vector.reciprocal(st[:, 6:7], st[:, 5:6])
                return E, st

            def emit_transp(E):
                # bf16 transposes, TPB 128x128 blocks packed into one PSUM
                # bank, evicted with a single cast alternating DVE/ACT so
                # neither engine's latency gates the PE.
                ET = etp.tile([P, NST, P], F32R, tag="ET")
                for grp in range(NTG):
                    tp = ps_small.tile([P, TPB, P], BF16, tag="small")
                    for j in range(TPB):
                        s_ = grp * TPB + j
                        nc.tensor.transpose(
                            tp[:, j, :], E[:, s_ * P : (s_ + 1) * P], ident[:]
                        )
                    dst = ET[:, grp * TPB : (grp + 1) * TPB, :]
                    if grp % 2 == 0:
                        nc.vector.tensor_copy(dst, tp[:])
                    else:
                        nc.scalar.copy(dst, tp[:])
                return ET

            def emit_out(ET, st, qt):
                for c in range(NSC):
                    cs = slice(c * NCH, (c + 1) * NCH)
                    po = ps_out.tile([P, NCH], F32, tag="po")
                    for s_ in range(NST):
                        nc.tensor.matmul(
                            po[:],
                            ET[:, s_, :],
                            enc_sb[s_][:, cs],
                            start=(s_ == 0),
                            stop=(s_ == NST - 1),
                        )
                    ob = outp.tile([P, NCH], F32, tag="ob")
                    nc.vector.tensor_scalar_mul(ob[:], po[:], st[:, 6:7])
                    nc.sync.dma_start(
                        out=out[qt * P : (qt + 1) * P, cs], in_=ob[:]
                    )

            # Software-pipelined emission: PE order per steady-state q-tile is
            # transp(i), [P1(g+1) at group boundary], scores(i+1), out(i) -- the
            # softmax of i+1 runs on DVE/ACT while PE is busy with out(i).
            # Front: interleave P1(0), enc loads, and scores(0) at d-tile
            # granularity. The front is DMA-bound (inp 2MB + W 8MB + enc 16MB
            # must land); striping wt[dt]/enc[dt] across BOTH HWDGE queues in
            # need-order lets the two streams land in parallel while PE chews
            # P1 and scores(0).
            # Queue split: enc rides sync (ungated, free-running), wt+inp
            # ride scalar (slot-gated, stop-and-go). Mixing them on one
            # queue head-of-line blocks the enc stream behind a gated wt
            # trigger.
            _mark(nc, "front")
            projs = {}
            inp_g0 = emit_inp_load(0, nc.scalar)
            projT0 = projp.tile([P, NDT, QG], F32R, tag="projT")
            projs[0] = projT0
            cur_sc = ps_sc.tile([P, S], F32, tag="sc")
            # enc resident tile, loaded in 4 big DMAs (32KB contiguous per
            # partition each) on the free-running sync queue
            enc_big = constp.tile([P, NDT, S], F32R, tag="enc")
            enc_sb = [enc_big[:, dt_, :] for dt_ in range(NDT)]
            # artificial write-after-write gate: the first enc DMA starts
            # only after inp has fully landed, so the 2MB inp load isn't
            # starved to ~100GB/s by the 4MB enc transfers (P1 is the only
            # PE work available for the first ~30us of the front)
            nc.vector.tensor_copy(
                enc_big[0:1, 0:NDT:4, 0:1], inp_g0[0:1, 0:NHT:2, 0:1]
            )

            def front_dt(dt_):
                emit_scores_dt(cur_sc, projs[0], 0, dt_)

            for dtp in range(NDT // 2):
                if dtp % 2 == 0:
                    gi = dtp // 2
                    nc.sync.dma_start(
                        out=enc_big[:, 4 * gi : 4 * gi + 4, :],
                        in_=r32(enc[:, 4 * gi : 4 * gi + 4, :]),
                    )
                emit_p1_pair(0, dtp, projs[0], inp_g0, nc.scalar, per_dt=front_dt)
            # prefetch group 1's inp right after the front: the dummy copy
            # chains it behind the last enc group (so it doesn't steal front
            # bandwidth) and behind P1(0)'s reads of the shared slot
            nc.vector.tensor_copy(inp_g0[0:1, 0, 0:1], enc_big[0:1, NDT - 1, 0:1])
            inp_g1 = emit_inp_load(1, nc.sync)
            # softmax(qt+1) is emitted right after scores(qt+1), BEFORE
            # out(qt): the DVE reduce-max runs at the start of out(qt)'s PE
            # window instead of queueing behind its eviction scales, so
            # transp(qt+1) never stalls on E. The last q-tile's softmax is
            # likewise hidden behind out(6).
            _mark(nc, "softmax(0)")
            E_cur, st_cur = emit_softmax(cur_sc)
            for qt in range(NQT):
                _mark(nc, f"transp({qt})")
                ET = emit_transp(E_cur)
                nxt = qt + 1
                st_next = None
                if nxt < NQT:
                    g, qloc = divmod(nxt, QTPG)
                    if qloc == 0:
                        # group boundary: stream W for group g, interleaving
                        # the next q-tile's scores at d-tile granularity so
                        # the PE never head-of-line blocks on the W DMA
                        _mark(nc, f"P1({g})+scores({nxt})")
                        inp_g = inp_g1
                        projT = projp.tile([P, NDT, QG], F32R, tag="projT")
                        projs[g] = projT
                        sc_new = ps_sc.tile([P, S], F32, tag="sc")
                        cur_sc = sc_new
                        for dtp in range(NDT // 2):
                            emit_p1_pair(
                                g,
                                dtp,
                                projT,
                                inp_g,
                                nc.sync,
                                per_dt=lambda dt_: emit_scores_dt(
                                    sc_new, projT, 0, dt_
                                ),
                            )
                    else:
                        _mark(nc, f"scores({nxt})")
                        cur_sc = emit_scores(projs[g], qloc)
                    _mark(nc, f"softmax({nxt})")
                    E_cur, st_next = emit_softmax(cur_sc)
                _mark(nc, f"out({qt})")
                emit_out(ET, st_cur, qt)
                st_cur = st_next
            _mark(nc, "end")
            loop_ctx.__exit__(None, None, None)

    nc.compile()
    return nc


PHASES = []  # (instruction id, label) marks populated during build, for tsim


def _mark(nc, label):
    nm = nc.get_next_instruction_name()  # burns one name; fine
    PHASES.append((int(nm.split("-")[1]), label))


_NC_CACHE = {}


def _get_program(loop_n: int = 1) -> bass.Bass:
    if loop_n not in _NC_CACHE:
        PHASES.clear()
        _NC_CACHE[loop_n] = build_program(loop_n)
    return _NC_CACHE[loop_n]


def _prep_in_maps(input, encoder_output, W, b):
    input = np.ascontiguousarray(input, dtype=np.float32)
    encoder_output = np.ascontiguousarray(encoder_output, dtype=np.float32)
    W = np.ascontiguousarray(W, dtype=np.float32)
    b = np.ascontiguousarray(b, dtype=np.float32)

    # inpT[b, g, p, ht, q] = input[b, g*QG+q, ht*P+p]  -- per partition p the
    # DMA row [NHT, QG] is contiguous (16KB)
    inpT = np.ascontiguousarray(
        input.reshape(B, NG, QG, NHT, P).transpose(0, 1, 4, 3, 2)
    )
    # wt[p, dt, ht, dj] = W[dt*P+dj, ht*P+p] -- per partition p the whole
    # [NDT, NHT, P] span is contiguous (64KB), so paired-dt DMAs read 8KB
    # contiguous per partition
    wt = np.ascontiguousarray(
        W.reshape(NDT, P, NHT, P).transpose(3, 0, 2, 1)
    )
    # enc_pre[b, p, dt, s] = enc[b, dt*P+p, s] -- 128KB contiguous/partition
    encP = np.ascontiguousarray(
        encoder_output.reshape(B, NDT, P, S).transpose(0, 2, 1, 3)
    )
    bvec = np.ascontiguousarray(b.reshape(NDT, P).T)  # [P, NDT]

    return [
        {"inpT": inpT[i], "wt": wt, "enc": encP[i], "bvec": bvec}
        for i in range(B)
    ]


def run(input, encoder_output, W, b, trace=False, loop_n=1):
    """Returns (out [B, TQ, S] float32, BassKernelResults)."""
    nc = _get_program(loop_n)
    in_maps = _prep_in_maps(input, encoder_output, W, b)
    res = run_bass_kernel_spmd(nc, in_maps, list(range(B)), trace=trace)
    out = np.stack([np.asarray(res.results[i]["out"]) for i in range(B)])
    return out, res


def kernel(input, encoder_output, W, b):
    out, _ = run(input, encoder_output, W, b, trace=False)
    return out


# revision 32
# speedup vs baseline: 1.1473x; 1.1394x over previous
"""Trainium2 Bass kernel for the batched attention module:

    proj   = input @ W.T + b            # [B, TQ, 2H]
    scores = proj @ enc                 # [B, TQ, S]   (enc: [B, 2H, S], S == 2H)
    attn   = softmax(scores, axis=-1)
    out    = attn @ enc                 # [B, TQ, S]

Sharding: data-parallel over batch, one batch per NeuronCore (8 cores).
All matmuls run as float32r (fp32 stored, fp22 multiplied, fp32 accumulated)
which streams at 1 cycle/row on the PE -- 4x the fp32 rate.

Dataflow per core (batch):
  P1:  projT[d,q] = sum_h WT[h,d] * inputT[h,q]  (+bias), per q-group of 512
  P2:  scores[q,s] (q on partitions) accumulated over 16 d-tiles in PSUM
       softmax stats on the free dim: DVE row-max (negated) -> ACT Exp with
       per-partition bias and accumulated row-sum -> DVE reciprocal.
       E is written in bf16: the PE transposes then run at 1 cyc/row
       (vs 2 for fp32) and the PSUM->SBUF eviction casts are 2x cheaper.
  T:   PE-transpose E=[q,s] in bf16, 4 128x128 blocks packed per PSUM
       bank, evicted with one cast per block-group alternating DVE/ACT
  P3:  out[q,v] = sum_s ET[s,q].T @ enc[s,v], scaled by 1/rowsum on eviction

DMA: the front (inp 2MB + W 8MB + enc 16MB) is striped across both HWDGE
queues (sync + scalar) in need-order so the two streams progress in
parallel; the second q-group's W re-stream rides the otherwise idle sync
queue mid-kernel. The scalar queue carries no DMA after the front, so the
ACT exp/copy instructions never queue behind descriptors.
"""

import sys

import numpy as np

for _p in ("/opt/trn_rl_repo",):
    if _p not in sys.path:
        sys.path.insert(0, _p)

from concourse import bacc, bass, mybir, tile  # noqa: E402
from concourse.bass_utils import run_bass_kernel_spmd  # noqa: E402
from concourse.masks import make_identity  # noqa: E402

F32 = mybir.dt.float32
F32R = mybir.dt.float32r
BF16 = mybir.dt.bfloat16
AF = mybir.ActivationFunctionType
AX = mybir.AxisListType
ALU = mybir.AluOpType

B = 8
TQ = 1024
H = 1024
D = 2 * H  # 2048, proj feature dim == contraction dim of scores
S = 2 * H  # 2048
P = 128

NHT = H // P  # 8  h-tiles
NDT = D // P  # 16 d-tiles
NST = S // P  # 16 s-tiles
NQT = TQ // P  # 8 q-tiles
QG = 512  # q-group width for the proj phase (moving dim >= 256 for f32r rate)
NG = TQ // QG  # 2 groups
QTPG = QG // P  # 4 q-tiles per group
NCH = 512  # moving-dim chunk for scores/out matmuls (one PSUM bank of fp32)
NSC = S // NCH  # 4
TPB = 8  # transposes packed per PSUM bank (8 x [128,128]bf16 = 2KB/partition)
NTG = NST // TPB  # 2 transpose groups per q-tile


def r32(ap):
    return ap.bitcast(F32R)


def build_program(loop_n: int = 1) -> bass.Bass:
    nc = bacc.Bacc(
        "TRN2",
        target_bir_lowering=False,
        debug=False,
        # default 16KB/partition of SWDGE descriptor scratch; we only use
        # HWDGE queues (sync/scalar), so reclaim most of it for tiles
        dynamic_dma_scratch_size=2048,
    )
    # host-side pre-transposed layouts: per SBUF partition the DMA reads one
    # long contiguous row (16KB for inp, 4KB for wt) -- short rows (<=2KB)
    # measured ~105GB/s vs ~250+GB/s for long rows
    inpT = nc.declare_dram_parameter("inpT", [NG, P, NHT, QG], F32, isOutput=False)
    wt = nc.declare_dram_parameter("wt", [P, NDT, NHT, P], F32, isOutput=False)
    enc = nc.declare_dram_parameter("enc", [P, NDT, S], F32, isOutput=False)
    bvec = nc.declare_dram_parameter("bvec", [P, NDT], F32, isOutput=False)
    out = nc.declare_dram_parameter("out", [TQ, S], F32, isOutput=True)

    with tile.TileContext(nc) as tc:
        with (
            tc.tile_pool(name="const", bufs=1) as constp,
            tc.tile_pool(name="inp", bufs=1) as inpp,
            tc.tile_pool(name="wtp", bufs=2) as wtp,
            tc.tile_pool(name="projp", bufs=1) as projp,
            tc.tile_pool(name="ep", bufs=1) as ep,
            tc.tile_pool(name="etp", bufs=2) as etp,
            tc.tile_pool(name="outp", bufs=1) as outp,
            tc.tile_pool(name="statp", bufs=2) as statp,
            tc.tile_pool(name="ps_sc", bufs=1, space="PSUM") as ps_sc,
            tc.tile_pool(name="ps_small", bufs=2, space="PSUM") as ps_small,
            tc.tile_pool(name="ps_out", bufs=2, space="PSUM") as ps_out,
        ):
            ident = constp.tile([P, P], BF16)
            make_identity(nc, ident[:])
            bias_sb = constp.tile([P, NDT], F32)
            nc.sync.dma_start(out=bias_sb[:], in_=bvec[:])

            import contextlib

            loop_ctx = (
                tc.For_i(0, loop_n, 1, hint_engines=(mybir.EngineType.PE,))
                if loop_n > 1
                else contextlib.nullcontext()
            )
            loop_ctx.__enter__()

            def emit_p1_pair(g, dtp, projT, inp_g, dma, per_dt=None):
                """Two d-tiles of the proj phase: one 1MB wt DMA + 2x(8
                matmuls + evict). Pairing halves the number of gated wt
                triggers so the stream stays ahead of the PE."""
                wt_sl = wtp.tile([P, 2, NHT, P], F32R, tag="wt")
                dma.dma_start(
                    out=wt_sl[:], in_=r32(wt[:, 2 * dtp : 2 * dtp + 2])
                )
                for k in range(2):
                    dt_ = 2 * dtp + k
                    pp = ps_small.tile([P, QG], F32, tag="small")
                    for ht in range(NHT):
                        nc.tensor.matmul(
                            pp[:],
                            wt_sl[:, k, ht, :],
                            inp_g[:, ht, :],
                            start=(ht == 0),
                            stop=(ht == NHT - 1),
                        )
                    # DVE, not ACT: activation instrs mixed with the DMA queue
                    # on the scalar engine measured ~40us each on HW
                    nc.vector.tensor_scalar_add(
                        projT[:, dt_, :], pp[:], bias_sb[:, dt_ : dt_ + 1]
                    )
                    if per_dt is not None:
                        per_dt(dt_)

            def emit_inp_load(g, dma):
                inp_g = inpp.tile([P, NHT, QG], F32R, tag="inp")
                dma.dma_start(out=inp_g[:], in_=r32(inpT[g]))
                return inp_g

            def emit_scores_dt(sc, projT, qloc, dt_):
                qs = slice(qloc * P, (qloc + 1) * P)
                for c in range(NSC):
                    cs = slice(c * NCH, (c + 1) * NCH)
                    nc.tensor.matmul(
                        sc[:, cs],
                        projT[:, dt_, qs],
                        enc_sb[dt_][:, cs],
                        start=(dt_ == 0),
                        stop=(dt_ == NDT - 1),
                    )

            def emit_scores(projT, qloc):
                # dt-outer so each enc tile unlocks 4 matmuls as it arrives
                # (4 interleaved PSUM accumulation groups, one per bank).
                sc = ps_sc.tile([P, S], F32, tag="sc")
                for dt_ in range(NDT):
                    emit_scores_dt(sc, projT, qloc, dt_)
                return sc

            def emit_softmax(sc):
                # split into s-halves: the first transposes only gate on the
                # first exp half, shortening the exposed chain after scores
                st = statp.tile([P, 8], F32, tag="stat")
                H2 = S // 2
                nc.vector.tensor_reduce(
                    st[:, 0:1], sc[:, 0:H2], axis=AX.X, op=ALU.max, negate=True
                )
                nc.vector.tensor_reduce(
                    st[:, 1:2], sc[:, H2:], axis=AX.X, op=ALU.max, negate=True
                )
                # -max_total = min(-max_lo, -max_hi)
                nc.vector.scalar_tensor_tensor(
                    st[:, 2:3], st[:, 0:1], 0.0, st[:, 1:2], ALU.add, ALU.min
                )
                E = ep.tile([P, S], BF16, tag="E")
                nc.scalar.activation(
                    E[:, 0:H2],
                    sc[:, 0:H2],
                    AF.Exp,
                    bias=st[:, 2:3],
                    scale=1.0,
                    accum_out=st[:, 3:4],
                )
                nc.scalar.activation(
                    E[:, H2:],
                    sc[:, H2:],
                    AF.Exp,
                    bias=st[:, 2:3],
                    scale=1.0,
                    accum_out=st[:, 4:5],
                )
                nc.vector.scalar_tensor_tensor(
                    st[:, 5:6], st[:, 3:4], 0.0, st[:, 4:5], ALU.add, ALU.add
                )
                nc.vector.reciprocal(st[:, 6:7], st[:, 5:6])
                return E, st

            def emit_transp(E):
                # bf16 transposes, TPB 128x128 blocks packed into one PSUM
                # bank, evicted with a single cast alternating DVE/ACT so
                # neither engine's latency gates the PE.
                ET = etp.tile([P, NST, P], F32R, tag="ET")
                for grp in range(NTG):
                    tp = ps_small.tile([P, TPB, P], BF16, tag="small")
                    for j in range(TPB):
                        s_ = grp * TPB + j
                        nc.tensor.transpose(
                            tp[:, j, :], E[:, s_ * P : (s_ + 1) * P], ident[:]
                        )
                    dst = ET[:, grp * TPB : (grp + 1) * TPB, :]
                    if grp % 2 == 0:
                        nc.vector.tensor_copy(dst, tp[:])
                    else:
                        nc.scalar.copy(dst, tp[:])
                return ET

            def emit_out(ET, st, qt):
                for c in range(NSC):
                    cs = slice(c * NCH, (c + 1) * NCH)
                    po = ps_out.tile([P, NCH], F32, tag="po")
                    for s_ in range(NST):
                        nc.tensor.matmul(
                            po[:],
                            ET[:, s_, :],
                            enc_sb[s_][:, cs],
                            start=(s_ == 0),
                            stop=(s_ == NST - 1),
                        )
                    ob = outp.tile([P, NCH], F32, tag="ob")
                    nc.vector.tensor_scalar_mul(ob[:], po[:], st[:, 6:7])
                    nc.sync.dma_start(
                        out=out[qt * P : (qt + 1) * P, cs], in_=ob[:]
                    )

            # Software-pipelined emission: PE order per steady-state q-tile is
            # transp(i), [P1(g+1) at group boundary], scores(i+1), out(i) -- the
            # softmax of i+1 runs on DVE/ACT while PE is busy with out(i).
            # Front: interleave P1(0), enc loads, and scores(0) at d-tile
            # granularity. The front is DMA-bound (inp 2MB + W 8MB + enc 16MB
            # must land); striping wt[dt]/enc[dt] across BOTH HWDGE queues in
            # need-order lets the two streams land in parallel while PE chews
            # P1 and scores(0).
            # Queue split: enc rides sync (ungated, free-running), wt+inp
            # ride scalar (slot-gated, stop-and-go). Mixing them on one
            # queue head-of-line blocks the enc stream behind a gated wt
            # trigger.
            _mark(nc, "front")
            projs = {}
            inp_g0 = emit_inp_load(0, nc.scalar)
            projT0 = projp.tile([P, NDT, QG], F32R, tag="projT")
            projs[0] = projT0
            cur_sc = ps_sc.tile([P, S], F32, tag="sc")
            # enc resident tile, loaded in 4 big DMAs (32KB contiguous per
            # partition each) on the free-running sync queue
            enc_big = constp.tile([P, NDT, S], F32R, tag="enc")
            enc_sb = [enc_big[:, dt_, :] for dt_ in range(NDT)]
            # artificial write-after-write gate: the first enc DMA starts
            # only after inp has fully landed, so the 2MB inp load isn't
            # starved to ~100GB/s by the 4MB enc transfers (P1 is the only
            # PE work available for the first ~30us of the front)
            nc.vector.tensor_copy(
                enc_big[0:1, 0:NDT:4, 0:1], inp_g0[0:1, 0:NHT:2, 0:1]
            )

            def front_dt(dt_):
                emit_scores_dt(cur_sc, projs[0], 0, dt_)

            for dtp in range(NDT // 2):
                if dtp % 2 == 0:
                    gi = dtp // 2
                    nc.sync.dma_start(
                        out=enc_big[:, 4 * gi : 4 * gi + 4, :],
                        in_=r32(enc[:, 4 * gi : 4 * gi + 4, :]),
                    )
                emit_p1_pair(0, dtp, projs[0], inp_g0, nc.scalar, per_dt=front_dt)
                if 1 <= dtp <= 5:
                    # HAM keep-warm: the front stalls on the enc/wt streams in
                    # 5-13us gaps, long enough for the PE clock gate to
                    # re-throttle to 1.2GHz. Park cheap dummy transposes (into
                    # the otherwise-idle ps_out banks, never read) in the gaps
                    # so the real matmuls resume at 2.4GHz.
                    warm = ps_out.tile([P, TPB, P], BF16, tag="po")
                    for j in range(24):
                        nc.tensor.transpose(warm[:, j % TPB, :], ident[:], ident[:])
            # prefetch group 1's inp right after the front: the dummy copy
            # chains it behind the last enc group (so it doesn't steal front
            # bandwidth) and behind P1(0)'s reads of the shared slot
            nc.vector.tensor_copy(inp_g0[0:1, 0, 0:1], enc_big[0:1, NDT - 1, 0:1])
            inp_g1 = emit_inp_load(1, nc.sync)
            # softmax(qt+1) is emitted right after scores(qt+1), BEFORE
            # out(qt): the DVE reduce-max runs at the start of out(qt)'s PE
            # window instead of queueing behind its eviction scales, so
            # transp(qt+1) never stalls on E. The last q-tile's softmax is
            # likewise hidden behind out(6).
            _mark(nc, "softmax(0)")
            E_cur, st_cur = emit_softmax(cur_sc)
            for qt in range(NQT):
                _mark(nc, f"transp({qt})")
                ET = emit_transp(E_cur)
                nxt = qt + 1
                st_next = None
                if nxt < NQT:
                    g, qloc = divmod(nxt, QTPG)
                    if qloc == 0:
                        # group boundary: stream W for group g, interleaving
                        # the next q-tile's scores at d-tile granularity so
                        # the PE never head-of-line blocks on the W DMA
                        _mark(nc, f"P1({g})+scores({nxt})")
                        inp_g = inp_g1
                        projT = projp.tile([P, NDT, QG], F32R, tag="projT")
                        projs[g] = projT
                        sc_new = ps_sc.tile([P, S], F32, tag="sc")
                        cur_sc = sc_new
                        for dtp in range(NDT // 2):
                            emit_p1_pair(
                                g,
                                dtp,
                                projT,
                                inp_g,
                                nc.sync,
                                per_dt=lambda dt_: emit_scores_dt(
                                    sc_new, projT, 0, dt_
                                ),
                            )
                    else:
                        _mark(nc, f"scores({nxt})")
                        cur_sc = emit_scores(projs[g], qloc)
                    _mark(nc, f"softmax({nxt})")
                    E_cur, st_next = emit_softmax(cur_sc)
                _mark(nc, f"out({qt})")
                emit_out(ET, st_cur, qt)
                st_cur = st_next
            _mark(nc, "end")
            loop_ctx.__exit__(None, None, None)

    nc.compile()
    return nc


PHASES = []  # (instruction id, label) marks populated during build, for tsim


def _mark(nc, label):
    nm = nc.get_next_instruction_name()  # burns one name; fine
    PHASES.append((int(nm.split("-")[1]), label))


_NC_CACHE = {}


def _get_program(loop_n: int = 1) -> bass.Bass:
    if loop_n not in _NC_CACHE:
        PHASES.clear()
        _NC_CACHE[loop_n] = build_program(loop_n)
    return _NC_CACHE[loop_n]


def _prep_in_maps(input, encoder_output, W, b):
    input = np.ascontiguousarray(input, dtype=np.float32)
    encoder_output = np.ascontiguousarray(encoder_output, dtype=np.float32)
    W = np.ascontiguousarray(W, dtype=np.float32)
    b = np.ascontiguousarray(b, dtype=np.float32)

    # inpT[b, g, p, ht, q] = input[b, g*QG+q, ht*P+p]  -- per partition p the
    # DMA row [NHT, QG] is contiguous (16KB)
    inpT = np.ascontiguousarray(
        input.reshape(B, NG, QG, NHT, P).transpose(0, 1, 4, 3, 2)
    )
    # wt[p, dt, ht, dj] = W[dt*P+dj, ht*P+p] -- per partition p the whole
    # [NDT, NHT, P] span is contiguous (64KB), so paired-dt DMAs read 8KB
    # contiguous per partition
    wt = np.ascontiguousarray(
        W.reshape(NDT, P, NHT, P).transpose(3, 0, 2, 1)
    )
    # enc_pre[b, p, dt, s] = enc[b, dt*P+p, s] -- 128KB contiguous/partition
    encP = np.ascontiguousarray(
        encoder_output.reshape(B, NDT, P, S).transpose(0, 2, 1, 3)
    )
    bvec = np.ascontiguousarray(b.reshape(NDT, P).T)  # [P, NDT]

    return [
        {"inpT": inpT[i], "wt": wt, "enc": encP[i], "bvec": bvec}
        for i in range(B)
    ]


def run(input, encoder_output, W, b, trace=False, loop_n=1):
    """Returns (out [B, TQ, S] float32, BassKernelResults)."""
    nc = _get_program(loop_n)
    in_maps = _prep_in_maps(input, encoder_output, W, b)
    res = run_bass_kernel_spmd(nc, in_maps, list(range(B)), trace=trace)
    out = np.stack([np.asarray(res.results[i]["out"]) for i in range(B)])
    return out, res


def kernel(input, encoder_output, W, b):
    out, _ = run(input, encoder_output, W, b, trace=False)
    return out


# revision 34
# speedup vs baseline: 1.1562x; 1.0077x over previous
"""Trainium2 Bass kernel for the batched attention module:

    proj   = input @ W.T + b            # [B, TQ, 2H]
    scores = proj @ enc                 # [B, TQ, S]   (enc: [B, 2H, S], S == 2H)
    attn   = softmax(scores, axis=-1)
    out    = attn @ enc                 # [B, TQ, S]

Sharding: data-parallel over batch, one batch per NeuronCore (8 cores).
All matmuls run as float32r (fp32 stored, fp22 multiplied, fp32 accumulated)
which streams at 1 cycle/row on the PE -- 4x the fp32 rate.

Dataflow per core (batch):
  P1:  projT[d,q] = sum_h WT[h,d] * inputT[h,q]  (+bias), per q-group of 512
  P2:  scores[q,s] (q on partitions) accumulated over 16 d-tiles in PSUM
       softmax stats on the free dim: DVE row-max (negated) -> ACT Exp with
       per-partition bias and accumulated row-sum -> DVE reciprocal.
       E is written in bf16: the PE transposes then run at 1 cyc/row
       (vs 2 for fp32) and the PSUM->SBUF eviction casts are 2x cheaper.
  T:   PE-transpose E=[q,s] in bf16, 8 128x128 blocks packed per PSUM
       bank, evicted with one cast per block-group alternating DVE/ACT
  P3:  out[q,v] = sum_s ET[s,q].T @ enc[s,v], scaled by 1/rowsum on eviction

DMA: the front (inp 2MB + W 8MB + enc 16MB) is striped across both HWDGE
queues (sync + scalar) in need-order so the two streams progress in
parallel; the second q-group's W re-stream rides the otherwise idle sync
queue mid-kernel. The scalar queue carries no DMA after the front, so the
ACT exp/copy instructions never queue behind descriptors.
"""

import sys

import numpy as np

for _p in ("/opt/trn_rl_repo",):
    if _p not in sys.path:
        sys.path.insert(0, _p)

from concourse import bacc, bass, mybir, tile  # noqa: E402
from concourse.bass_utils import run_bass_kernel_spmd  # noqa: E402
from concourse.masks import make_identity  # noqa: E402

F32 = mybir.dt.float32
F32R = mybir.dt.float32r
BF16 = mybir.dt.bfloat16
AF = mybir.ActivationFunctionType
AX = mybir.AxisListType
ALU = mybir.AluOpType

B = 8
TQ = 1024
H = 1024
D = 2 * H  # 2048, proj feature dim == contraction dim of scores
S = 2 * H  # 2048
P = 128

NHT = H // P  # 8  h-tiles
NDT = D // P  # 16 d-tiles
NST = S // P  # 16 s-tiles
NQT = TQ // P  # 8 q-tiles
QG = 512  # q-group width for the proj phase (moving dim >= 256 for f32r rate)
NG = TQ // QG  # 2 groups
QTPG = QG // P  # 4 q-tiles per group
NCH = 512  # moving-dim chunk for scores/out matmuls (one PSUM bank of fp32)
NSC = S // NCH  # 4
TPB = 8  # transposes packed per PSUM bank (8 x [128,128]bf16 = 2KB/partition)
NTG = NST // TPB  # 2 transpose groups per q-tile


def r32(ap):
    return ap.bitcast(F32R)


def build_program(loop_n: int = 1) -> bass.Bass:
    nc = bacc.Bacc(
        "TRN2",
        target_bir_lowering=False,
        debug=False,
        # default 16KB/partition of SWDGE descriptor scratch; we only use
        # HWDGE queues (sync/scalar), so reclaim most of it for tiles
        dynamic_dma_scratch_size=2048,
    )
    # host-side pre-transposed layouts: per SBUF partition the DMA reads one
    # long contiguous row (16KB for inp, 4KB for wt) -- short rows (<=2KB)
    # measured ~105GB/s vs ~250+GB/s for long rows
    inpT = nc.declare_dram_parameter("inpT", [NG, P, NHT, QG], F32, isOutput=False)
    wt = nc.declare_dram_parameter("wt", [P, NDT, NHT, P], F32, isOutput=False)
    enc = nc.declare_dram_parameter("enc", [P, NDT, S], F32, isOutput=False)
    bvec = nc.declare_dram_parameter("bvec", [P, NDT], F32, isOutput=False)
    out = nc.declare_dram_parameter("out", [TQ, S], F32, isOutput=True)

    with tile.TileContext(nc) as tc:
        with (
            tc.tile_pool(name="const", bufs=1) as constp,
            tc.tile_pool(name="inp", bufs=1) as inpp,
            tc.tile_pool(name="wtp", bufs=2) as wtp,
            tc.tile_pool(name="projp", bufs=1) as projp,
            tc.tile_pool(name="ep", bufs=1) as ep,
            tc.tile_pool(name="etp", bufs=2) as etp,
            tc.tile_pool(name="outp", bufs=1) as outp,
            tc.tile_pool(name="statp", bufs=2) as statp,
            tc.tile_pool(name="ps_sc", bufs=1, space="PSUM") as ps_sc,
            tc.tile_pool(name="ps_small", bufs=2, space="PSUM") as ps_small,
            tc.tile_pool(name="ps_out", bufs=2, space="PSUM") as ps_out,
        ):
            ident = constp.tile([P, P], BF16)
            make_identity(nc, ident[:])
            bias_sb = constp.tile([P, NDT], F32)
            nc.sync.dma_start(out=bias_sb[:], in_=bvec[:])

            import contextlib

            loop_ctx = (
                tc.For_i(0, loop_n, 1, hint_engines=(mybir.EngineType.PE,))
                if loop_n > 1
                else contextlib.nullcontext()
            )
            loop_ctx.__enter__()

            def emit_p1_pair(g, dtp, projT, inp_g, dma, per_dt=None):
                """Two d-tiles of the proj phase: one 1MB wt DMA + 2x(8
                matmuls + evict). Pairing halves the number of gated wt
                triggers so the stream stays ahead of the PE."""
                wt_sl = wtp.tile([P, 2, NHT, P], F32R, tag="wt")
                dma.dma_start(
                    out=wt_sl[:], in_=r32(wt[:, 2 * dtp : 2 * dtp + 2])
                )
                for k in range(2):
                    dt_ = 2 * dtp + k
                    pp = ps_small.tile([P, QG], F32, tag="small")
                    for ht in range(NHT):
                        nc.tensor.matmul(
                            pp[:],
                            wt_sl[:, k, ht, :],
                            inp_g[:, ht, :],
                            start=(ht == 0),
                            stop=(ht == NHT - 1),
                        )
                    # DVE, not ACT: activation instrs mixed with the DMA queue
                    # on the scalar engine measured ~40us each on HW
                    nc.vector.tensor_scalar_add(
                        projT[:, dt_, :], pp[:], bias_sb[:, dt_ : dt_ + 1]
                    )
                    if per_dt is not None:
                        per_dt(dt_)

            def emit_inp_load(g, dma):
                inp_g = inpp.tile([P, NHT, QG], F32R, tag="inp")
                dma.dma_start(out=inp_g[:], in_=r32(inpT[g]))
                return inp_g

            def emit_scores_dt(sc, projT, qloc, dt_):
                qs = slice(qloc * P, (qloc + 1) * P)
                for c in range(NSC):
                    cs = slice(c * NCH, (c + 1) * NCH)
                    nc.tensor.matmul(
                        sc[:, cs],
                        projT[:, dt_, qs],
                        enc_sb[dt_][:, cs],
                        start=(dt_ == 0),
                        stop=(dt_ == NDT - 1),
                    )

            def emit_scores(projT, qloc):
                # dt-outer so each enc tile unlocks 4 matmuls as it arrives
                # (4 interleaved PSUM accumulation groups, one per bank).
                sc = ps_sc.tile([P, S], F32, tag="sc")
                for dt_ in range(NDT):
                    emit_scores_dt(sc, projT, qloc, dt_)
                return sc

            def emit_softmax(sc):
                # split into s-halves: the first transposes only gate on the
                # first exp half, shortening the exposed chain after scores
                st = statp.tile([P, 8], F32, tag="stat")
                H2 = S // 2
                nc.vector.tensor_reduce(
                    st[:, 0:1], sc[:, 0:H2], axis=AX.X, op=ALU.max, negate=True
                )
                nc.vector.tensor_reduce(
                    st[:, 1:2], sc[:, H2:], axis=AX.X, op=ALU.max, negate=True
                )
                # -max_total = min(-max_lo, -max_hi)
                nc.vector.scalar_tensor_tensor(
                    st[:, 2:3], st[:, 0:1], 0.0, st[:, 1:2], ALU.add, ALU.min
                )
                E = ep.tile([P, S], BF16, tag="E")
                nc.scalar.activation(
                    E[:, 0:H2],
                    sc[:, 0:H2],
                    AF.Exp,
                    bias=st[:, 2:3],
                    scale=1.0,
                    accum_out=st[:, 3:4],
                )
                nc.scalar.activation(
                    E[:, H2:],
                    sc[:, H2:],
                    AF.Exp,
                    bias=st[:, 2:3],
                    scale=1.0,
                    accum_out=st[:, 4:5],
                )
                nc.vector.scalar_tensor_tensor(
                    st[:, 5:6], st[:, 3:4], 0.0, st[:, 4:5], ALU.add, ALU.add
                )
                nc.vector.reciprocal(st[:, 6:7], st[:, 5:6])
                return E, st

            def emit_transp(E):
                # bf16 transposes, TPB 128x128 blocks packed into one PSUM
                # bank, evicted with a single cast alternating DVE/ACT so
                # neither engine's latency gates the PE.
                ET = etp.tile([P, NST, P], F32R, tag="ET")
                for grp in range(NTG):
                    tp = ps_small.tile([P, TPB, P], BF16, tag="small")
                    for j in range(TPB):
                        s_ = grp * TPB + j
                        nc.tensor.transpose(
                            tp[:, j, :], E[:, s_ * P : (s_ + 1) * P], ident[:]
                        )
                    dst = ET[:, grp * TPB : (grp + 1) * TPB, :]
                    if grp % 2 == 0:
                        nc.vector.tensor_copy(dst, tp[:])
                    else:
                        nc.scalar.copy(dst, tp[:])
                return ET

            def emit_out(ET, st, qt):
                for c in range(NSC):
                    cs = slice(c * NCH, (c + 1) * NCH)
                    po = ps_out.tile([P, NCH], F32, tag="po")
                    for s_ in range(NST):
                        nc.tensor.matmul(
                            po[:],
                            ET[:, s_, :],
                            enc_sb[s_][:, cs],
                            start=(s_ == 0),
                            stop=(s_ == NST - 1),
                        )
                    ob = outp.tile([P, NCH], F32, tag="ob")
                    nc.vector.tensor_scalar_mul(ob[:], po[:], st[:, 6:7])
                    nc.sync.dma_start(
                        out=out[qt * P : (qt + 1) * P, cs], in_=ob[:]
                    )

            # Software-pipelined emission: PE order per steady-state q-tile is
            # transp(i), [P1(g+1) at group boundary], scores(i+1), out(i) -- the
            # softmax of i+1 runs on DVE/ACT while PE is busy with out(i).
            # Front: interleave P1(0), enc loads, and scores(0) at d-tile
            # granularity. The front is DMA-bound (inp 2MB + W 8MB + enc 16MB
            # must land); striping wt[dt]/enc[dt] across BOTH HWDGE queues in
            # need-order lets the two streams land in parallel while PE chews
            # P1 and scores(0).
            # Queue split: enc rides sync (ungated, free-running), wt+inp
            # ride scalar (slot-gated, stop-and-go). Mixing them on one
            # queue head-of-line blocks the enc stream behind a gated wt
            # trigger.
            _mark(nc, "front")
            projs = {}
            inp_g0 = emit_inp_load(0, nc.scalar)
            projT0 = projp.tile([P, NDT, QG], F32R, tag="projT")
            projs[0] = projT0
            cur_sc = ps_sc.tile([P, S], F32, tag="sc")
            # enc resident tile, loaded in 4 big DMAs (32KB contiguous per
            # partition each) on the free-running sync queue
            enc_big = constp.tile([P, NDT, S], F32R, tag="enc")
            enc_sb = [enc_big[:, dt_, :] for dt_ in range(NDT)]
            # artificial write-after-write gate: the first enc DMA starts
            # only after inp has fully landed, so the 2MB inp load isn't
            # starved to ~100GB/s by the 4MB enc transfers (P1 is the only
            # PE work available for the first ~30us of the front)
            nc.vector.tensor_copy(
                enc_big[0:1, 0:NDT:4, 0:1], inp_g0[0:1, 0:NHT:2, 0:1]
            )

            def front_dt(dt_):
                emit_scores_dt(cur_sc, projs[0], 0, dt_)

            for dtp in range(NDT // 2):
                if dtp % 2 == 0:
                    gi = dtp // 2
                    nc.sync.dma_start(
                        out=enc_big[:, 4 * gi : 4 * gi + 4, :],
                        in_=r32(enc[:, 4 * gi : 4 * gi + 4, :]),
                    )
                emit_p1_pair(0, dtp, projs[0], inp_g0, nc.scalar, per_dt=front_dt)
            # prefetch group 1's inp right after the front: the dummy copy
            # chains it behind the last enc group (so it doesn't steal front
            # bandwidth) and behind P1(0)'s reads of the shared slot
            nc.vector.tensor_copy(inp_g0[0:1, 0, 0:1], enc_big[0:1, NDT - 1, 0:1])
            inp_g1 = emit_inp_load(1, nc.sync)
            # softmax(qt+1) is emitted right after scores(qt+1), BEFORE
            # out(qt): the DVE reduce-max runs at the start of out(qt)'s PE
            # window instead of queueing behind its eviction scales, so
            # transp(qt+1) never stalls on E. The last q-tile's softmax is
            # likewise hidden behind out(6).
            _mark(nc, "softmax(0)")
            E_cur, st_cur = emit_softmax(cur_sc)
            for qt in range(NQT):
                _mark(nc, f"transp({qt})")
                ET = emit_transp(E_cur)
                nxt = qt + 1
                st_next = None
                if nxt < NQT:
                    g, qloc = divmod(nxt, QTPG)
                    if qloc == 0:
                        # group boundary: stream W for group g, interleaving
                        # the next q-tile's scores at d-tile granularity so
                        # the PE never head-of-line blocks on the W DMA
                        _mark(nc, f"P1({g})+scores({nxt})")
                        inp_g = inp_g1
                        projT = projp.tile([P, NDT, QG], F32R, tag="projT")
                        projs[g] = projT
                        sc_new = ps_sc.tile([P, S], F32, tag="sc")
                        cur_sc = sc_new
                        for dtp in range(NDT // 2):
                            emit_p1_pair(
                                g,
                                dtp,
                                projT,
                                inp_g,
                                nc.sync,
                                per_dt=lambda dt_: emit_scores_dt(
                                    sc_new, projT, 0, dt_
                                ),
                            )
                    else:
                        _mark(nc, f"scores({nxt})")
                        cur_sc = emit_scores(projs[g], qloc)
                    _mark(nc, f"softmax({nxt})")
                    E_cur, st_next = emit_softmax(cur_sc)
                _mark(nc, f"out({qt})")
                emit_out(ET, st_cur, qt)
                st_cur = st_next
            _mark(nc, "end")
            loop_ctx.__exit__(None, None, None)

    nc.compile()
    return nc


PHASES = []  # (instruction id, label) marks populated during build, for tsim


def _mark(nc, label):
    nm = nc.get_next_instruction_name()  # burns one name; fine
    PHASES.append((int(nm.split("-")[1]), label))


_NC_CACHE = {}


def _get_program(loop_n: int = 1) -> bass.Bass:
    if loop_n not in _NC_CACHE:
        PHASES.clear()
        _NC_CACHE[loop_n] = build_program(loop_n)
    return _NC_CACHE[loop_n]


def _prep_in_maps(input, encoder_output, W, b):
    input = np.ascontiguousarray(input, dtype=np.float32)
    encoder_output = np.ascontiguousarray(encoder_output, dtype=np.float32)
    W = np.ascontiguousarray(W, dtype=np.float32)
    b = np.ascontiguousarray(b, dtype=np.float32)

    # inpT[b, g, p, ht, q] = input[b, g*QG+q, ht*P+p]  -- per partition p the
    # DMA row [NHT, QG] is contiguous (16KB)
    inpT = np.ascontiguousarray(
        input.reshape(B, NG, QG, NHT, P).transpose(0, 1, 4, 3, 2)
    )
    # wt[p, dt, ht, dj] = W[dt*P+dj, ht*P+p] -- per partition p the whole
    # [NDT, NHT, P] span is contiguous (64KB), so paired-dt DMAs read 8KB
    # contiguous per partition
    wt = np.ascontiguousarray(
        W.reshape(NDT, P, NHT, P).transpose(3, 0, 2, 1)
    )
    # enc_pre[b, p, dt, s] = enc[b, dt*P+p, s] -- 128KB contiguous/partition
    encP = np.ascontiguousarray(
        encoder_output.reshape(B, NDT, P, S).transpose(0, 2, 1, 3)
    )
    bvec = np.ascontiguousarray(b.reshape(NDT, P).T)  # [P, NDT]

    return [
        {"inpT": inpT[i], "wt": wt, "enc": encP[i], "bvec": bvec}
        for i in range(B)
    ]


def run(input, encoder_output, W, b, trace=False, loop_n=1):
    """Returns (out [B, TQ, S] float32, BassKernelResults)."""
    nc = _get_program(loop_n)
    in_maps = _prep_in_maps(input, encoder_output, W, b)
    res = run_bass_kernel_spmd(nc, in_maps, list(range(B)), trace=trace)
    out = np.stack([np.asarray(res.results[i]["out"]) for i in range(B)])
    return out, res


def kernel(input, encoder_output, W, b):
    out, _ = run(input, encoder_output, W, b, trace=False)
    return out
